# revision 89
# baseline (speedup 1.0000x reference)
"""Trainium2 Bass kernel for a 2-layer GATv2 (DGL-style) over a random graph.

Self-contained: takes FULL inputs (as produced by the problem's setup_inputs),
shards across 8 NeuronCores internally, returns the FULL output [N, 64] f32.

Strategy (per core, dst-sharded):
 - Each core owns N/8 consecutive dst nodes and all edges pointing to them.
 - Layer tables fs/fd are built by on-device matmuls from a host-transposed
   x shard; fs tables are AllGather'd so each core can gather any src row.
   Layer-2 tables pack [fs2|fd2] per 256B bf16 row.
 - Edges are sorted by dst, grouped into 128-dst-node groups, split into two
   src buckets (src < 25000 / >= 25000) so `dma_gather`'s int16 indices can
   address the table, and padded to fixed per-(group,bucket) tile counts so
   one compiled program serves all cores.
 - src-row gathers are SWDGE dma_gathers round-robined over 4 queues so
   transfers drain in parallel (single-queue drain is the baseline limit).
 - fd rows are NOT gathered: a group's dst rows are one affine 128-row load;
   per-edge fd replication is a PE matmul with a host-streamed transposed
   one-hot (fp8), accumulated with an identity matmul of fs into PSUM so
   u = fs[src]+fd[dst] never touches the vector engine.
 - Per 128-edge tile: leaky = max(0.2u (scalar engine), u); logits = per-
   head dot with attn (DVE mult+reduce); a = exp(logits) (edge softmax
   numerator; max-subtraction dropped — logits are O(1)); w = a * fs[src].
 - Scatter/softmax-denominator: host-streamed fp8 one-hot matmul
   accumulating [w | a] into PSUM per group; normalize by the summed a.
"""

import math

import numpy as np
import ml_dtypes

import concourse.bass as bass
import concourse.mybir as mybir
import concourse.tile as tile
from concourse import bacc
from concourse._compat import axon_active

P = 128
F32 = mybir.dt.float32
BF16 = mybir.dt.bfloat16
FP8 = mybir.dt.float8e4
I16 = mybir.dt.int16

NEG_SLOPE = 0.2
DEN_EPS = 1e-20


class Cfg:
    def __init__(self, N=50000, E=800000, F_IN=128, H1=4, D1=32, OUT=64, NC=8,
                 SG=4, NSPLIT=25000):
        self.N, self.E, self.F_IN, self.H1, self.D1, self.OUT, self.NC = \
            N, E, F_IN, H1, D1, OUT, NC
        self.D1TOT = H1 * D1              # 128
        self.NSPLIT = NSPLIT              # src bucket split (< 32768)
        self.NLOC = N // NC
        self.NG = math.ceil(self.NLOC / P)
        self.NPAD = self.NG * P
        self.SG = SG                      # groups per supergroup
        # filled by preprocess:
        self.T_A = None                   # tiles per group, bucket A
        self.T_B = None


def _wrap_idx(arr_i16):
    """[n] int16 -> [128, n/16] idx tile layout (16-partition wrap, 8x rep)."""
    n = arr_i16.shape[0]
    assert n % 16 == 0
    idx16 = arr_i16.reshape(-1, 16).T           # [16, n/16]
    return np.tile(idx16, (8, 1)).copy()        # [128, n/16]


def _slot_cols(arr, width=P):
    """[n] -> [128, n/128]: slot s -> partition s%128, col s//128."""
    return np.ascontiguousarray(arr.reshape(-1, width).T)


def preprocess(inputs, cfg: Cfg):
    """Build per-core input maps + fill cfg.T_A/T_B."""
    x = np.asarray(inputs["x"], np.float32)
    src = np.asarray(inputs["src"], np.int64)
    dst = np.asarray(inputs["dst"], np.int64)
    N, NC, NLOC, NG, NSPLIT = cfg.N, cfg.NC, cfg.NLOC, cfg.NG, cfg.NSPLIT

    per_core = []
    maxA = maxB = 1
    for c in range(NC):
        m = (dst >= c * NLOC) & (dst < (c + 1) * NLOC)
        es, ed = src[m], dst[m] - c * NLOC
        g = ed // P
        b = (es >= NSPLIT).astype(np.int64)
        order = np.lexsort((ed, b, g))
        es, ed, g, b = es[order], ed[order], g[order], b[order]
        # counts per (g, bucket)
        key = g * 2 + b
        cnt = np.bincount(key, minlength=NG * 2).reshape(NG, 2)
        maxA = max(maxA, int(cnt[:, 0].max()))
        maxB = max(maxB, int(cnt[:, 1].max()))
        per_core.append((es, ed, g, b, cnt))

    T_A = math.ceil(maxA / P)
    T_B = math.ceil(maxB / P)
    cfg.T_A, cfg.T_B = T_A, T_B
    NT = T_A + T_B
    GSL = NT * P                          # unified slots per group

    # weight/const tensors (identical on all cores)
    Wl1 = np.asarray(inputs["Wl1"], np.float32)
    Wr1 = np.asarray(inputs["Wr1"], np.float32)
    bl1 = np.asarray(inputs["bl1"], np.float32)
    br1 = np.asarray(inputs["br1"], np.float32)
    attn1 = np.asarray(inputs["attn1"], np.float32)
    Wl2 = np.asarray(inputs["Wl2"], np.float32)
    Wr2 = np.asarray(inputs["Wr2"], np.float32)
    bl2 = np.asarray(inputs["bl2"], np.float32)
    br2 = np.asarray(inputs["br2"], np.float32)
    attn2 = np.asarray(inputs["attn2"], np.float32)

    W1cat = np.concatenate([Wl1, Wr1], axis=1)            # [F_IN, 2*D1TOT]
    b1rep = np.tile(np.concatenate([bl1, br1])[None, :], (P, 1))
    W2cat = np.concatenate([Wl2, Wr2], axis=1).astype(ml_dtypes.bfloat16)
    b2rep = np.tile(np.concatenate([bl2, br2])[None, :], (P, 1)).astype(
        ml_dtypes.bfloat16)
    attn1_rep = np.tile(attn1.reshape(1, -1), (P, 1)).astype(ml_dtypes.bfloat16)
    attn2_rep = np.tile(attn2.reshape(1, -1), (P, 1)).astype(ml_dtypes.bfloat16)
    ident_bf = np.eye(P, dtype=ml_dtypes.bfloat16)

    in_maps = []
    for c in range(NC):
        es, ed, g, b, cnt = per_core[c]
        # rank of each edge within its (g, bucket) run
        startsA = np.zeros(NG, np.int64)
        startsB = np.zeros(NG, np.int64)
        run_start = np.concatenate([[0], np.cumsum(cnt.reshape(-1))[:-1]])
        key = g * 2 + b
        rank = np.arange(es.shape[0]) - run_start[key]
        # unified slot (group-major, A slots then B slots)
        slot_u = g * GSL + b * (T_A * P) + rank
        # bucket-stream slots
        slotA = g * (T_A * P) + rank      # valid where b==0
        slotB = g * (T_B * P) + rank      # valid where b==1

        fsA = np.zeros(NG * T_A * P, np.int16)
        fsB = np.zeros(NG * T_B * P, np.int16)
        fsA[slotA[b == 0]] = es[b == 0].astype(np.int16)
        fsB[slotB[b == 1]] = (es[b == 1] - NSPLIT).astype(np.int16)
        dstslot = np.full(NG * GSL, 255, np.int16)
        dstslot[slot_u] = (ed - g * P).astype(np.int16)
        # host-built one-hot scatter matrices: tile (g,t) partition p ->
        # column dstslot (255 sentinel rows are all-zero = padding)
        oh = (dstslot.reshape(NG * NT, P)[:, :, None]
              == np.arange(P, dtype=np.int16)[None, None, :])
        soh = np.ascontiguousarray(
            oh.transpose(1, 0, 2).reshape(P, NG * NT * P)
        ).astype(ml_dtypes.float8_e4m3)
        # transposed one-hot: lhsT for the fd row-replicate matmul
        # (fd_e[p] = fd_g[dstslot[p]])
        sohT = np.ascontiguousarray(
            oh.transpose(2, 0, 1).reshape(P, NG * NT * P)
        ).astype(ml_dtypes.float8_e4m3)

        xT = np.zeros((cfg.F_IN, cfg.NPAD), np.float32)
        xT[:, :NLOC] = x[c * NLOC:(c + 1) * NLOC].T

        in_maps.append({
            "xT": xT,
            "W1cat": W1cat, "b1rep": b1rep,
            "W2cat": np.asarray(W2cat), "b2rep": np.asarray(b2rep),
            "attn1_rep": np.asarray(attn1_rep),
            "attn2_rep": np.asarray(attn2_rep),
            "ident_bf": np.asarray(ident_bf),
            "idxA": _wrap_idx(fsA), "idxB": _wrap_idx(fsB),
            "sonehot": np.asarray(soh),
            "sonehotT": np.asarray(sohT),
        })
    return in_maps


def build_program(cfg: Cfg, debug=False):
    nc = bacc.Bacc("TRN2", target_bir_lowering=False, debug=debug,
                   num_devices=cfg.NC, num_swdge_queues=4)
    N, NG, NPAD, NLOC = cfg.N, cfg.NG, cfg.NPAD, cfg.NLOC
    T_A, T_B, SG, NSPLIT = cfg.T_A, cfg.T_B, cfg.SG, cfg.NSPLIT
    F_IN, D1TOT, H1, D1, OUT = cfg.F_IN, cfg.D1TOT, cfg.H1, cfg.D1, cfg.OUT
    NT = T_A + T_B
    core_ids = list(range(cfg.NC))

    # ---- parameters ----
    par = {}
    def param(name, shape, dtype):
        par[name] = nc.declare_dram_parameter(name, list(shape), dtype,
                                              isOutput=False)
        return par[name]

    xT = param("xT", (F_IN, NPAD), F32)
    W1cat = param("W1cat", (F_IN, 2 * D1TOT), F32)
    b1rep = param("b1rep", (P, 2 * D1TOT), F32)
    W2cat = param("W2cat", (D1TOT, 2 * OUT), BF16)
    b2rep = param("b2rep", (P, 2 * OUT), BF16)
    attn1_rep = param("attn1_rep", (P, D1TOT), BF16)
    attn2_rep = param("attn2_rep", (P, OUT), BF16)
    ident_bf = param("ident_bf", (P, P), BF16)
    idxA = param("idxA", (P, NG * T_A * 8), I16)
    idxB = param("idxB", (P, NG * T_B * 8), I16)
    sonehot = param("sonehot", (P, NG * NT * P), FP8)
    sonehotT = param("sonehotT", (P, NG * NT * P), FP8)

    out_local = nc.declare_dram_parameter("out_local", [NPAD, OUT], F32,
                                          isOutput=True)

    # ---- internal DRAM ----
    fs1_local = nc.dram_tensor("fs1_local", [NPAD, D1TOT], BF16)
    fd1_local = nc.dram_tensor("fd1_local", [NPAD, D1TOT], BF16)
    fs1_full = nc.dram_tensor("fs1_full", [N, D1TOT], BF16, addr_space="Shared")
    # gather in_ap offsets are broken on HW -> separate upper-half table
    fs1_hi = nc.dram_tensor("fs1_hi", [N - NSPLIT, D1TOT], BF16)
    # layer-2 tables pack [fs2 | fd2] per row (256B bf16): src gathers use the
    # fs half, dst gathers the fd half — bf16 rate with no wasted gather bytes
    fs2_local = nc.dram_tensor("fs2_local", [NPAD, 2 * OUT], BF16)
    fs2_full = nc.dram_tensor("fs2_full", [N, 2 * OUT], BF16,
                              addr_space="Shared")
    fs2_hi = nc.dram_tensor("fs2_hi", [N - NSPLIT, 2 * OUT], BF16)

    supergroups = [(s, min(s + SG, NG)) for s in range(0, NG, SG)]

    with tile.TileContext(nc) as tc:
        with (
            tc.tile_pool(name="const", bufs=1) as cpool,
        ):
            # constants resident for the whole kernel
            c_attn1 = cpool.tile([P, D1TOT], BF16)
            nc.sync.dma_start(out=c_attn1[:], in_=attn1_rep[:, :])
            c_attn2 = cpool.tile([P, OUT], BF16)
            nc.sync.dma_start(out=c_attn2[:], in_=attn2_rep[:, :])
            c_ident = cpool.tile([P, P], BF16)
            nc.sync.dma_start(out=c_ident[:], in_=ident_bf[:, :])
            c_W2 = cpool.tile([D1TOT, 2 * OUT], BF16)
            nc.sync.dma_start(out=c_W2[:], in_=W2cat[:, :])
            c_b2r = cpool.tile([P, 2 * OUT], BF16)
            nc.sync.dma_start(out=c_b2r[:], in_=b2rep[:, :])
            c_idxA = cpool.tile([P, NG * T_A * 8], I16)
            nc.sync.dma_start(out=c_idxA[:], in_=idxA[:, :])
            c_idxB = cpool.tile([P, NG * T_B * 8], I16)
            nc.sync.dma_start(out=c_idxB[:], in_=idxB[:, :])

            # ================= phase A: layer-1 node tables =================
            with (
                tc.tile_pool(name="ph0", bufs=1) as p0,
                tc.tile_pool(name="ph0w", bufs=6) as p0w,
                tc.tile_pool(name="psA", bufs=4, space="PSUM") as psA,
            ):
                c_W1 = p0.tile([F_IN, 2 * D1TOT], F32)
                nc.sync.dma_start(out=c_W1[:], in_=W1cat[:, :])
                c_xT = p0.tile([F_IN, NPAD], F32)
                # chunked so group-0 matmuls start before the whole load lands
                xt_step = 8 * P
                for x0 in range(0, NPAD, xt_step):
                    x1 = min(NPAD, x0 + xt_step)
                    nc.sync.dma_start(out=c_xT[:, x0:x1], in_=xT[:, x0:x1])
                c_b1r = p0.tile([P, 2 * D1TOT], F32)
                nc.sync.dma_start(out=c_b1r[:], in_=b1rep[:, :])
                for g in range(NG):
                    ps = psA.tile([P, 2 * D1TOT], F32)
                    nc.tensor.matmul(out=ps[:], lhsT=c_xT[:, g * P:(g + 1) * P],
                                     rhs=c_W1[:], start=True, stop=True)
                    sb = p0w.tile([P, 2 * D1TOT], BF16, tag="t1sb")
                    nc.vector.tensor_tensor(out=sb[:], in0=ps[:], in1=c_b1r[:],
                                            op=mybir.AluOpType.add)
                    nc.sync.dma_start(out=fs1_local[g * P:(g + 1) * P, :],
                                      in_=sb[:, 0:D1TOT])
                    nc.sync.dma_start(out=fd1_local[g * P:(g + 1) * P, :],
                                      in_=sb[:, D1TOT:2 * D1TOT])

            tc.strict_bb_all_engine_barrier()
            nc.gpsimd.collective_compute(
                "AllGather", mybir.AluOpType.bypass,
                replica_groups=[core_ids],
                ins=[fs1_local[0:NLOC, :]], outs=[fs1_full[:, :]],
            )
            tc.strict_bb_all_engine_barrier()
            # no barrier after: only B-bucket gathers depend on the hi copy
            nc.sync.dma_start(out=fs1_hi[:, :], in_=fs1_full[NSPLIT:N, :])

            # ============== phase B: layer-1 edges + layer-2 tables =========
            with (
                tc.tile_pool(name="hT", bufs=1) as hTp,
            ):
                c_hT = hTp.tile([D1TOT, NPAD], BF16)
                _edge_phase(
                    nc, tc, cfg, layer=1,
                    table_full=fs1_full, table_hi=fs1_hi, table_fd=fd1_local,
                    c_idxA=c_idxA, c_idxB=c_idxB,
                    sonehot=sonehot, sonehotT=sonehotT,
                    c_attn=c_attn1, feat=D1TOT, nheads=H1, hdim=D1,
                    supergroups=supergroups,
                    c_ident=c_ident, c_hT=c_hT, c_W2=c_W2, c_b2r=c_b2r,
                    fs2cat=fs2_local,
                    out_local=None,
                )

                tc.strict_bb_all_engine_barrier()
                nc.gpsimd.collective_compute(
                    "AllGather", mybir.AluOpType.bypass,
                    replica_groups=[core_ids],
                    ins=[fs2_local[0:NLOC, :]], outs=[fs2_full[:, :]],
                )
                tc.strict_bb_all_engine_barrier()
                nc.sync.dma_start(out=fs2_hi[:, :], in_=fs2_full[NSPLIT:N, :])

            # ================= phase D: layer-2 edges =======================
            _edge_phase(
                nc, tc, cfg, layer=2,
                table_full=fs2_full, table_hi=fs2_hi, table_fd=fs2_local,
                c_idxA=c_idxA, c_idxB=c_idxB,
                sonehot=sonehot, sonehotT=sonehotT,
                c_attn=c_attn2, feat=OUT, nheads=1, hdim=OUT,
                supergroups=supergroups,
                c_ident=c_ident, c_hT=None, c_W2=None, c_b2r=None,
                fs2cat=None,
                out_local=out_local,
                gcols=2 * OUT, fd_off=OUT,
            )

    nc.compile()
    return nc


def _edge_phase(nc, tc, cfg, layer, table_full, table_hi, table_fd,
                c_idxA, c_idxB, sonehot, sonehotT, c_attn,
                feat, nheads, hdim, supergroups,
                c_ident, c_hT, c_W2, c_b2r,
                fs2cat, out_local, gcols=None, fd_off=0):
    """Shared edge-processing loop for both layers.

    gcols: gathered row width in elements (>= feat; layer 2 rows pack
    [fs|fd]).  fd_off: column offset of the fd half within FD-gathered rows.
    """
    T_A, T_B, NSPLIT, N = cfg.T_A, cfg.T_B, cfg.NSPLIT, cfg.N
    NT = T_A + T_B
    CH = max(T_A, T_B)          # tiles per DVE chunk (whole bucket)
    dt_e = BF16                 # gathered edge-table dtype
    if gcols is None:
        gcols = feat
    wcols = feat + nheads       # [w | a]

    MAXI = getattr(cfg, "MAX_GATHER_IDX", 768)
    qrr = getattr(cfg, "_qrr", None) or [0]
    cfg._qrr = qrr

    def gather_chunked(out_tile, src_ap, idx_tile, col0, n_idx):
        """Issue dma_gather in <=MAXI-index chunks (HW desc-ring limit),
        round-robining the 4 SWDGE queues so transfers drain in parallel."""
        done = 0
        while done < n_idx:
            n = min(MAXI, n_idx - done)
            nc.gpsimd.dma_gather(
                out_ap=out_tile[:, (done // P):((done + n) // P), :],
                in_ap=src_ap,
                idxs_ap=idx_tile[:, col0 + done // 16:col0 + (done + n) // 16],
                num_idxs=n, num_idxs_reg=n, elem_size=gcols,
                queue_num=qrr[0] % 4)
            qrr[0] += 1
            done += n

    with (
        tc.tile_pool(name=f"g{layer}", bufs=2) as gp,
        tc.tile_pool(name=f"wk{layer}", bufs=4) as wk,
        tc.tile_pool(name=f"sm{layer}", bufs=6) as sm,
        tc.tile_pool(name=f"ps{layer}", bufs=2, space="PSUM") as pp,
        tc.tile_pool(name=f"pt{layer}", bufs=1, space="PSUM") as pt,
        tc.tile_pool(name=f"pf{layer}", bufs=1, space="PSUM") as pf,
    ):
        for (g0, g1) in supergroups:
            sgn = g1 - g0
            ntA, ntB, ntU = sgn * T_A, sgn * T_B, sgn * NT
            # one-hot + fd-block DMAs first: PE replicate work can begin
            # while the gathers stream in
            soh = gp.tile([P, ntU, P], FP8, tag="soh")
            nc.sync.dma_start(
                out=soh[:],
                in_=sonehot[:, g0 * NT * P:g1 * NT * P])
            sohT = gp.tile([P, ntU, P], FP8, tag="sohT")
            nc.sync.dma_start(
                out=sohT[:],
                in_=sonehotT[:, g0 * NT * P:g1 * NT * P])
            # the fd rows of this supergroup's dst nodes: one affine load
            fd_blk = gp.tile([P, sgn, gcols], dt_e, tag="fdb")
            nc.sync.dma_start(
                out=fd_blk[:],
                in_=table_fd[g0 * P:g1 * P, :].rearrange(
                    "(g p) c -> p g c", p=P))
            gA = gp.tile([P, ntA, gcols], dt_e, tag="gA")
            gather_chunked(gA, table_full[:, :], c_idxA, g0 * T_A * 8, ntA * P)
            gB = gp.tile([P, ntB, gcols], dt_e, tag="gB")
            gather_chunked(gB, table_hi[:, :], c_idxB, g0 * T_B * 8, ntB * P)

            for g in range(g0, g1):
                i = g - g0
                psg = pp.tile([P, wcols], F32, tag="scat")
                # per-bucket: PE builds u = fd-replicate + fs in PSUM, then
                # Scalar 0.2*u and DVE max write halves of the unified t_lr
                t_lr = wk.tile([P, NT, feat], dt_e, tag="tlr")
                for (buf, bbase, tcnt, boff) in (
                    (gA, i * T_A, T_A, 0),
                    (gB, i * T_B, T_B, T_A),
                ):
                    u_ps = pf.tile([P, CH, feat], F32, tag="fdp")
                    for t in range(tcnt):
                        nc.tensor.matmul(
                            out=u_ps[:, t, :],
                            lhsT=sohT[:, i * NT + boff + t, :],
                            rhs=fd_blk[:, i, fd_off:fd_off + feat],
                            start=True, stop=False)
                        nc.tensor.matmul(
                            out=u_ps[:, t, :],
                            lhsT=c_ident[:],
                            rhs=buf[:, bbase + t, 0:feat],
                            start=False, stop=True)
                    # leaky = max(0.2*u, u); DVE may read only one PSUM
                    # operand, so Scalar scales into SBUF first
                    us = wk.tile([P, CH, feat], dt_e, tag="us")
                    nc.scalar.activation(
                        out=us[:, 0:tcnt, :], in_=u_ps[:, 0:tcnt, :],
                        func=mybir.ActivationFunctionType.Copy,
                        scale=NEG_SLOPE)
                    nc.vector.tensor_tensor(
                        out=t_lr[:, boff:boff + tcnt, :],
                        in0=us[:, 0:tcnt, :],
                        in1=u_ps[:, 0:tcnt, :], op=mybir.AluOpType.max)
                # whole-group ops on the unified [A|B] tile
                tp = wk.tile([P, NT, feat], dt_e, tag="tp")
                nc.vector.tensor_tensor(
                    out=tp[:], in0=t_lr[:],
                    in1=c_attn[:].unsqueeze(1).to_broadcast([P, NT, feat]),
                    op=mybir.AluOpType.mult)
                lg = sm.tile([P, NT * nheads], F32, tag="lg")
                nc.vector.tensor_reduce(
                    out=lg[:].rearrange("p (n h) -> p n h", n=NT),
                    in_=tp[:].rearrange("p n (h d) -> p n h d", h=nheads),
                    axis=mybir.AxisListType.X, op=mybir.AluOpType.add)
                w = wk.tile([P, NT, wcols], dt_e, tag="w")
                nc.scalar.activation(
                    out=w[:, :, feat:wcols],
                    in_=lg[:].rearrange("p (n h) -> p n h", n=NT),
                    func=mybir.ActivationFunctionType.Exp)
                for (buf, bbase, tcnt, boff) in (
                    (gA, i * T_A, T_A, 0),
                    (gB, i * T_B, T_B, T_A),
                ):
                    fs_ch = buf[:, bbase:bbase + tcnt, 0:feat]
                    nc.vector.tensor_tensor(
                        out=w[:, boff:boff + tcnt, 0:feat].rearrange(
                            "p n (h d) -> p n h d", h=nheads),
                        in0=fs_ch.rearrange("p n (h d) -> p n h d",
                                            h=nheads),
                        in1=w[:, boff:boff + tcnt, feat:wcols].unsqueeze(3)
                            .to_broadcast([P, tcnt, nheads, hdim]),
                        op=mybir.AluOpType.mult)
                for t in range(NT):
                    nc.tensor.matmul(
                        out=psg[:], lhsT=soh[:, i * NT + t, :],
                        rhs=w[:, t, :],
                        start=(t == 0), stop=(t == NT - 1))

                # normalize group g
                den = sm.tile([P, nheads], F32, tag="den")
                nc.vector.tensor_scalar_add(
                    out=den[:], in0=psg[:, feat:wcols], scalar1=DEN_EPS)
                denr = sm.tile([P, nheads], F32, tag="denr")
                nc.vector.reciprocal(out=denr[:], in_=den[:])
                if layer == 1:
                    h_g = wk.tile([P, feat], BF16, tag="hg")
                    nc.vector.scalar_tensor_tensor(
                        out=h_g[:].rearrange("p (h d) -> p h d", h=nheads),
                        in0=psg[:, 0:feat].rearrange("p (h d) -> p h d",
                                                     h=nheads),
                        scalar=0.0, op0=mybir.AluOpType.max,
                        in1=denr[:].unsqueeze(2).to_broadcast(
                            [P, nheads, hdim]),
                        op1=mybir.AluOpType.mult)
                    # transpose into the feat-major layer-1 output
                    ps_t = pt.tile([P, P], BF16, tag="pst")
                    nc.tensor.transpose(out=ps_t[:], in_=h_g[:],
                                        identity=c_ident[:])
                    nc.scalar.activation(
                        out=c_hT[:, g * P:(g + 1) * P], in_=ps_t[:],
                        func=mybir.ActivationFunctionType.Copy)
                    # layer-2 tables for this group
                    ps2 = pt.tile([P, 2 * cfg.OUT], F32, tag="ps2")
                    nc.tensor.matmul(out=ps2[:],
                                     lhsT=c_hT[:, g * P:(g + 1) * P],
                                     rhs=c_W2[:], start=True, stop=True)
                    sb2 = wk.tile([P, 2 * cfg.OUT], BF16, tag="sb2")
                    nc.vector.tensor_tensor(out=sb2[:], in0=ps2[:],
                                            in1=c_b2r[:],
                                            op=mybir.AluOpType.add)
                    nc.sync.dma_start(out=fs2cat[g * P:(g + 1) * P, :],
                                      in_=sb2[:])
                else:
                    o_g = wk.tile([P, feat], F32, tag="og")
                    nc.vector.tensor_tensor(
                        out=o_g[:], in0=psg[:, 0:feat],
                        in1=denr[:].to_broadcast([P, feat]),
                        op=mybir.AluOpType.mult)
                    nc.sync.dma_start(out=out_local[g * P:(g + 1) * P, :],
                                      in_=o_g[:])


def assemble(results, cfg: Cfg):
    outs = [np.asarray(results[c]["out_local"])[:cfg.NLOC]
            for c in range(cfg.NC)]
    return np.concatenate(outs, axis=0).astype(np.float32)


def kernel(**inputs):
    from concourse.bass_utils import run_bass_kernel_spmd
    cfg = Cfg()
    in_maps = preprocess(inputs, cfg)
    nc = build_program(cfg, debug=not axon_active())
    res = run_bass_kernel_spmd(nc, in_maps, list(range(cfg.NC)))
    return assemble(res.results, cfg)



# revision 93
# speedup vs baseline: 1.0165x; 1.0165x over previous
"""Trainium2 Bass kernel for a 2-layer GATv2 (DGL-style) over a random graph.

Self-contained: takes FULL inputs (as produced by the problem's setup_inputs),
shards across 8 NeuronCores internally, returns the FULL output [N, 64] f32.

Strategy (per core, dst-sharded):
 - Each core owns N/8 consecutive dst nodes and all edges pointing to them.
 - Layer tables fs/fd are built by on-device matmuls from a host-transposed
   x shard; fs tables are AllGather'd so each core can gather any src row.
   Layer-2 tables pack [fs2|fd2] per 256B bf16 row.
 - Edges are sorted by dst, grouped into 128-dst-node groups, split into two
   src buckets (src < 25000 / >= 25000) so `dma_gather`'s int16 indices can
   address the table, and padded to fixed per-(group,bucket) tile counts so
   one compiled program serves all cores.
 - src-row gathers are SWDGE dma_gathers round-robined over 4 queues so
   transfers drain in parallel (single-queue drain is the baseline limit).
 - fd rows are NOT gathered: a group's dst rows are one affine 128-row load;
   per-edge fd replication is a PE matmul with a host-streamed transposed
   one-hot (fp8), accumulated with an identity matmul of fs into PSUM so
   u = fs[src]+fd[dst] never touches the vector engine.
 - Per 128-edge tile: leaky = max(0.2u (scalar engine), u); logits = per-
   head dot with attn (DVE mult+reduce); a = exp(logits) (edge softmax
   numerator; max-subtraction dropped — logits are O(1)); w = a * fs[src].
 - Scatter/softmax-denominator: host-streamed fp8 one-hot matmul
   accumulating [w | a] into PSUM per group; normalize by the summed a.
"""

import math

import numpy as np
import ml_dtypes

import concourse.bass as bass
import concourse.mybir as mybir
import concourse.tile as tile
from concourse import bacc
from concourse._compat import axon_active

P = 128
F32 = mybir.dt.float32
BF16 = mybir.dt.bfloat16
FP8 = mybir.dt.float8e4
I16 = mybir.dt.int16

NEG_SLOPE = 0.2
DEN_EPS = 1e-20


class Cfg:
    def __init__(self, N=50000, E=800000, F_IN=128, H1=4, D1=32, OUT=64, NC=8,
                 SG=4, NSPLIT=25000):
        self.N, self.E, self.F_IN, self.H1, self.D1, self.OUT, self.NC = \
            N, E, F_IN, H1, D1, OUT, NC
        self.D1TOT = H1 * D1              # 128
        self.NSPLIT = NSPLIT              # src bucket split (< 32768)
        self.NLOC = N // NC
        self.NG = math.ceil(self.NLOC / P)
        self.NPAD = self.NG * P
        self.SG = SG                      # groups per supergroup
        # filled by preprocess:
        self.T_A = None                   # tiles per group, bucket A
        self.T_B = None


def _wrap_idx(arr_i16):
    """[n] int16 -> [128, n/16] idx tile layout (16-partition wrap, 8x rep)."""
    n = arr_i16.shape[0]
    assert n % 16 == 0
    idx16 = arr_i16.reshape(-1, 16).T           # [16, n/16]
    return np.tile(idx16, (8, 1)).copy()        # [128, n/16]


def _slot_cols(arr, width=P):
    """[n] -> [128, n/128]: slot s -> partition s%128, col s//128."""
    return np.ascontiguousarray(arr.reshape(-1, width).T)


def preprocess(inputs, cfg: Cfg):
    """Build per-core input maps + fill cfg.T_A/T_B."""
    x = np.asarray(inputs["x"], np.float32)
    src = np.asarray(inputs["src"], np.int64)
    dst = np.asarray(inputs["dst"], np.int64)
    N, NC, NLOC, NG, NSPLIT = cfg.N, cfg.NC, cfg.NLOC, cfg.NG, cfg.NSPLIT

    per_core = []
    maxA = maxB = 1
    for c in range(NC):
        m = (dst >= c * NLOC) & (dst < (c + 1) * NLOC)
        es, ed = src[m], dst[m] - c * NLOC
        g = ed // P
        b = (es >= NSPLIT).astype(np.int64)
        order = np.lexsort((ed, b, g))
        es, ed, g, b = es[order], ed[order], g[order], b[order]
        # counts per (g, bucket)
        key = g * 2 + b
        cnt = np.bincount(key, minlength=NG * 2).reshape(NG, 2)
        maxA = max(maxA, int(cnt[:, 0].max()))
        maxB = max(maxB, int(cnt[:, 1].max()))
        per_core.append((es, ed, g, b, cnt))

    T_A = math.ceil(maxA / P)
    T_B = math.ceil(maxB / P)
    cfg.T_A, cfg.T_B = T_A, T_B
    NT = T_A + T_B
    GSL = NT * P                          # unified slots per group

    # weight/const tensors (identical on all cores)
    Wl1 = np.asarray(inputs["Wl1"], np.float32)
    Wr1 = np.asarray(inputs["Wr1"], np.float32)
    bl1 = np.asarray(inputs["bl1"], np.float32)
    br1 = np.asarray(inputs["br1"], np.float32)
    attn1 = np.asarray(inputs["attn1"], np.float32)
    Wl2 = np.asarray(inputs["Wl2"], np.float32)
    Wr2 = np.asarray(inputs["Wr2"], np.float32)
    bl2 = np.asarray(inputs["bl2"], np.float32)
    br2 = np.asarray(inputs["br2"], np.float32)
    attn2 = np.asarray(inputs["attn2"], np.float32)

    W1cat = np.concatenate([Wl1, Wr1], axis=1).astype(
        ml_dtypes.bfloat16)                               # [F_IN, 2*D1TOT]
    b1rep = np.tile(np.concatenate([bl1, br1])[None, :], (P, 1))
    W2cat = np.concatenate([Wl2, Wr2], axis=1).astype(ml_dtypes.bfloat16)
    b2rep = np.tile(np.concatenate([bl2, br2])[None, :], (P, 1)).astype(
        ml_dtypes.bfloat16)
    attn1_rep = np.tile(attn1.reshape(1, -1), (P, 1)).astype(ml_dtypes.bfloat16)
    attn2_rep = np.tile(attn2.reshape(1, -1), (P, 1)).astype(ml_dtypes.bfloat16)
    ident_bf = np.eye(P, dtype=ml_dtypes.bfloat16)

    in_maps = []
    for c in range(NC):
        es, ed, g, b, cnt = per_core[c]
        # rank of each edge within its (g, bucket) run
        startsA = np.zeros(NG, np.int64)
        startsB = np.zeros(NG, np.int64)
        run_start = np.concatenate([[0], np.cumsum(cnt.reshape(-1))[:-1]])
        key = g * 2 + b
        rank = np.arange(es.shape[0]) - run_start[key]
        # unified slot (group-major, A slots then B slots)
        slot_u = g * GSL + b * (T_A * P) + rank
        # bucket-stream slots
        slotA = g * (T_A * P) + rank      # valid where b==0
        slotB = g * (T_B * P) + rank      # valid where b==1

        fsA = np.zeros(NG * T_A * P, np.int16)
        fsB = np.zeros(NG * T_B * P, np.int16)
        fsA[slotA[b == 0]] = es[b == 0].astype(np.int16)
        fsB[slotB[b == 1]] = (es[b == 1] - NSPLIT).astype(np.int16)
        dstslot = np.full(NG * GSL, 255, np.int16)
        dstslot[slot_u] = (ed - g * P).astype(np.int16)
        # host-built one-hot scatter matrices: tile (g,t) partition p ->
        # column dstslot (255 sentinel rows are all-zero = padding)
        oh = (dstslot.reshape(NG * NT, P)[:, :, None]
              == np.arange(P, dtype=np.int16)[None, None, :])
        soh = np.ascontiguousarray(
            oh.transpose(1, 0, 2).reshape(P, NG * NT * P)
        ).astype(ml_dtypes.float8_e4m3)
        # transposed one-hot: lhsT for the fd row-replicate matmul
        # (fd_e[p] = fd_g[dstslot[p]])
        sohT = np.ascontiguousarray(
            oh.transpose(2, 0, 1).reshape(P, NG * NT * P)
        ).astype(ml_dtypes.float8_e4m3)

        xT = np.zeros((cfg.F_IN, cfg.NPAD), ml_dtypes.bfloat16)
        xT[:, :NLOC] = x[c * NLOC:(c + 1) * NLOC].T.astype(ml_dtypes.bfloat16)

        in_maps.append({
            "xT": xT,
            "W1cat": W1cat, "b1rep": b1rep,
            "W2cat": np.asarray(W2cat), "b2rep": np.asarray(b2rep),
            "attn1_rep": np.asarray(attn1_rep),
            "attn2_rep": np.asarray(attn2_rep),
            "ident_bf": np.asarray(ident_bf),
            "idxA": _wrap_idx(fsA), "idxB": _wrap_idx(fsB),
            "sonehot": np.asarray(soh),
            "sonehotT": np.asarray(sohT),
        })
    return in_maps


def build_program(cfg: Cfg, debug=False):
    nc = bacc.Bacc("TRN2", target_bir_lowering=False, debug=debug,
                   num_devices=cfg.NC, num_swdge_queues=4)
    N, NG, NPAD, NLOC = cfg.N, cfg.NG, cfg.NPAD, cfg.NLOC
    T_A, T_B, SG, NSPLIT = cfg.T_A, cfg.T_B, cfg.SG, cfg.NSPLIT
    F_IN, D1TOT, H1, D1, OUT = cfg.F_IN, cfg.D1TOT, cfg.H1, cfg.D1, cfg.OUT
    NT = T_A + T_B
    core_ids = list(range(cfg.NC))

    # ---- parameters ----
    par = {}
    def param(name, shape, dtype):
        par[name] = nc.declare_dram_parameter(name, list(shape), dtype,
                                              isOutput=False)
        return par[name]

    xT = param("xT", (F_IN, NPAD), BF16)
    W1cat = param("W1cat", (F_IN, 2 * D1TOT), BF16)
    b1rep = param("b1rep", (P, 2 * D1TOT), F32)
    W2cat = param("W2cat", (D1TOT, 2 * OUT), BF16)
    b2rep = param("b2rep", (P, 2 * OUT), BF16)
    attn1_rep = param("attn1_rep", (P, D1TOT), BF16)
    attn2_rep = param("attn2_rep", (P, OUT), BF16)
    ident_bf = param("ident_bf", (P, P), BF16)
    idxA = param("idxA", (P, NG * T_A * 8), I16)
    idxB = param("idxB", (P, NG * T_B * 8), I16)
    sonehot = param("sonehot", (P, NG * NT * P), FP8)
    sonehotT = param("sonehotT", (P, NG * NT * P), FP8)

    out_local = nc.declare_dram_parameter("out_local", [NPAD, OUT], F32,
                                          isOutput=True)

    # ---- internal DRAM ----
    fs1_local = nc.dram_tensor("fs1_local", [NPAD, D1TOT], BF16)
    fd1_local = nc.dram_tensor("fd1_local", [NPAD, D1TOT], BF16)
    fs1_full = nc.dram_tensor("fs1_full", [N, D1TOT], BF16, addr_space="Shared")
    # gather in_ap offsets are broken on HW -> separate upper-half table
    fs1_hi = nc.dram_tensor("fs1_hi", [N - NSPLIT, D1TOT], BF16)
    # layer-2 tables pack [fs2 | fd2] per row (256B bf16): src gathers use the
    # fs half, dst gathers the fd half — bf16 rate with no wasted gather bytes
    fs2_local = nc.dram_tensor("fs2_local", [NPAD, 2 * OUT], BF16)
    fs2_full = nc.dram_tensor("fs2_full", [N, 2 * OUT], BF16,
                              addr_space="Shared")
    fs2_hi = nc.dram_tensor("fs2_hi", [N - NSPLIT, 2 * OUT], BF16)

    supergroups = [(s, min(s + SG, NG)) for s in range(0, NG, SG)]

    with tile.TileContext(nc) as tc:
        with (
            tc.tile_pool(name="const", bufs=1) as cpool,
        ):
            # constants resident for the whole kernel
            c_attn1 = cpool.tile([P, D1TOT], BF16)
            nc.sync.dma_start(out=c_attn1[:], in_=attn1_rep[:, :])
            c_attn2 = cpool.tile([P, OUT], BF16)
            nc.sync.dma_start(out=c_attn2[:], in_=attn2_rep[:, :])
            c_ident = cpool.tile([P, P], BF16)
            nc.sync.dma_start(out=c_ident[:], in_=ident_bf[:, :])
            c_W2 = cpool.tile([D1TOT, 2 * OUT], BF16)
            nc.sync.dma_start(out=c_W2[:], in_=W2cat[:, :])
            c_b2r = cpool.tile([P, 2 * OUT], BF16)
            nc.sync.dma_start(out=c_b2r[:], in_=b2rep[:, :])
            c_idxA = cpool.tile([P, NG * T_A * 8], I16)
            nc.sync.dma_start(out=c_idxA[:], in_=idxA[:, :])
            c_idxB = cpool.tile([P, NG * T_B * 8], I16)
            nc.sync.dma_start(out=c_idxB[:], in_=idxB[:, :])

            # ================= phase A: layer-1 node tables =================
            with (
                tc.tile_pool(name="ph0", bufs=1) as p0,
                tc.tile_pool(name="ph0w", bufs=6) as p0w,
                tc.tile_pool(name="psA", bufs=4, space="PSUM") as psA,
            ):
                c_W1 = p0.tile([F_IN, 2 * D1TOT], BF16)
                nc.sync.dma_start(out=c_W1[:], in_=W1cat[:, :])
                c_xT = p0.tile([F_IN, NPAD], BF16)
                # chunked so group-0 matmuls start before the whole load lands
                xt_step = 8 * P
                for x0 in range(0, NPAD, xt_step):
                    x1 = min(NPAD, x0 + xt_step)
                    nc.sync.dma_start(out=c_xT[:, x0:x1], in_=xT[:, x0:x1])
                c_b1r = p0.tile([P, 2 * D1TOT], F32)
                nc.sync.dma_start(out=c_b1r[:], in_=b1rep[:, :])
                for g in range(NG):
                    ps = psA.tile([P, 2 * D1TOT], F32)
                    nc.tensor.matmul(out=ps[:], lhsT=c_xT[:, g * P:(g + 1) * P],
                                     rhs=c_W1[:], start=True, stop=True)
                    sb = p0w.tile([P, 2 * D1TOT], BF16, tag="t1sb")
                    nc.vector.tensor_tensor(out=sb[:], in0=ps[:], in1=c_b1r[:],
                                            op=mybir.AluOpType.add)
                    nc.sync.dma_start(out=fs1_local[g * P:(g + 1) * P, :],
                                      in_=sb[:, 0:D1TOT])
                    nc.sync.dma_start(out=fd1_local[g * P:(g + 1) * P, :],
                                      in_=sb[:, D1TOT:2 * D1TOT])

            tc.strict_bb_all_engine_barrier()
            nc.gpsimd.collective_compute(
                "AllGather", mybir.AluOpType.bypass,
                replica_groups=[core_ids],
                ins=[fs1_local[0:NLOC, :]], outs=[fs1_full[:, :]],
            )
            tc.strict_bb_all_engine_barrier()
            # no barrier after: only B-bucket gathers depend on the hi copy
            nc.sync.dma_start(out=fs1_hi[:, :], in_=fs1_full[NSPLIT:N, :])

            # ============== phase B: layer-1 edges + layer-2 tables =========
            with (
                tc.tile_pool(name="hT", bufs=1) as hTp,
            ):
                c_hT = hTp.tile([D1TOT, NPAD], BF16)
                _edge_phase(
                    nc, tc, cfg, layer=1,
                    table_full=fs1_full, table_hi=fs1_hi, table_fd=fd1_local,
                    c_idxA=c_idxA, c_idxB=c_idxB,
                    sonehot=sonehot, sonehotT=sonehotT,
                    c_attn=c_attn1, feat=D1TOT, nheads=H1, hdim=D1,
                    supergroups=supergroups,
                    c_ident=c_ident, c_hT=c_hT, c_W2=c_W2, c_b2r=c_b2r,
                    fs2cat=fs2_local,
                    out_local=None,
                )

                tc.strict_bb_all_engine_barrier()
                nc.gpsimd.collective_compute(
                    "AllGather", mybir.AluOpType.bypass,
                    replica_groups=[core_ids],
                    ins=[fs2_local[0:NLOC, :]], outs=[fs2_full[:, :]],
                )
                tc.strict_bb_all_engine_barrier()
                nc.sync.dma_start(out=fs2_hi[:, :], in_=fs2_full[NSPLIT:N, :])

            # ================= phase D: layer-2 edges =======================
            _edge_phase(
                nc, tc, cfg, layer=2,
                table_full=fs2_full, table_hi=fs2_hi, table_fd=fs2_local,
                c_idxA=c_idxA, c_idxB=c_idxB,
                sonehot=sonehot, sonehotT=sonehotT,
                c_attn=c_attn2, feat=OUT, nheads=1, hdim=OUT,
                supergroups=supergroups,
                c_ident=c_ident, c_hT=None, c_W2=None, c_b2r=None,
                fs2cat=None,
                out_local=out_local,
                gcols=2 * OUT, fd_off=OUT,
            )

    nc.compile()
    return nc


def _edge_phase(nc, tc, cfg, layer, table_full, table_hi, table_fd,
                c_idxA, c_idxB, sonehot, sonehotT, c_attn,
                feat, nheads, hdim, supergroups,
                c_ident, c_hT, c_W2, c_b2r,
                fs2cat, out_local, gcols=None, fd_off=0):
    """Shared edge-processing loop for both layers.

    gcols: gathered row width in elements (>= feat; layer 2 rows pack
    [fs|fd]).  fd_off: column offset of the fd half within FD-gathered rows.
    """
    T_A, T_B, NSPLIT, N = cfg.T_A, cfg.T_B, cfg.NSPLIT, cfg.N
    NT = T_A + T_B
    CH = max(T_A, T_B)          # tiles per DVE chunk (whole bucket)
    dt_e = BF16                 # gathered edge-table dtype
    if gcols is None:
        gcols = feat
    wcols = feat + nheads       # [w | a]

    MAXI = getattr(cfg, "MAX_GATHER_IDX", 768)
    qrr = getattr(cfg, "_qrr", None) or [0]
    cfg._qrr = qrr

    def gather_chunked(out_tile, src_ap, idx_tile, col0, n_idx):
        """Issue dma_gather in <=MAXI-index chunks (HW desc-ring limit),
        round-robining the 4 SWDGE queues so transfers drain in parallel."""
        done = 0
        while done < n_idx:
            n = min(MAXI, n_idx - done)
            nc.gpsimd.dma_gather(
                out_ap=out_tile[:, (done // P):((done + n) // P), :],
                in_ap=src_ap,
                idxs_ap=idx_tile[:, col0 + done // 16:col0 + (done + n) // 16],
                num_idxs=n, num_idxs_reg=n, elem_size=gcols,
                queue_num=qrr[0] % 4)
            qrr[0] += 1
            done += n

    with (
        tc.tile_pool(name=f"g{layer}", bufs=2) as gp,
        tc.tile_pool(name=f"wk{layer}", bufs=4) as wk,
        tc.tile_pool(name=f"sm{layer}", bufs=6) as sm,
        tc.tile_pool(name=f"ps{layer}", bufs=2, space="PSUM") as pp,
        tc.tile_pool(name=f"pt{layer}", bufs=1, space="PSUM") as pt,
        tc.tile_pool(name=f"pf{layer}", bufs=1, space="PSUM") as pf,
    ):
        for (g0, g1) in supergroups:
            sgn = g1 - g0
            ntA, ntB, ntU = sgn * T_A, sgn * T_B, sgn * NT
            # one-hot + fd-block DMAs first: PE replicate work can begin
            # while the gathers stream in
            soh = gp.tile([P, ntU, P], FP8, tag="soh")
            nc.sync.dma_start(
                out=soh[:],
                in_=sonehot[:, g0 * NT * P:g1 * NT * P])
            sohT = gp.tile([P, ntU, P], FP8, tag="sohT")
            nc.sync.dma_start(
                out=sohT[:],
                in_=sonehotT[:, g0 * NT * P:g1 * NT * P])
            # the fd rows of this supergroup's dst nodes: one affine load
            fd_blk = gp.tile([P, sgn, gcols], dt_e, tag="fdb")
            nc.sync.dma_start(
                out=fd_blk[:],
                in_=table_fd[g0 * P:g1 * P, :].rearrange(
                    "(g p) c -> p g c", p=P))
            gA = gp.tile([P, ntA, gcols], dt_e, tag="gA")
            gather_chunked(gA, table_full[:, :], c_idxA, g0 * T_A * 8, ntA * P)
            gB = gp.tile([P, ntB, gcols], dt_e, tag="gB")
            gather_chunked(gB, table_hi[:, :], c_idxB, g0 * T_B * 8, ntB * P)

            for g in range(g0, g1):
                i = g - g0
                psg = pp.tile([P, wcols], F32, tag="scat")
                # per-bucket: PE builds u = fd-replicate + fs in PSUM, then
                # Scalar 0.2*u and DVE max write halves of the unified t_lr
                t_lr = wk.tile([P, NT, feat], dt_e, tag="tlr")
                for (buf, bbase, tcnt, boff) in (
                    (gA, i * T_A, T_A, 0),
                    (gB, i * T_B, T_B, T_A),
                ):
                    u_ps = pf.tile([P, CH, feat], F32, tag="fdp")
                    for t in range(tcnt):
                        nc.tensor.matmul(
                            out=u_ps[:, t, :],
                            lhsT=sohT[:, i * NT + boff + t, :],
                            rhs=fd_blk[:, i, fd_off:fd_off + feat],
                            start=True, stop=False)
                        nc.tensor.matmul(
                            out=u_ps[:, t, :],
                            lhsT=c_ident[:],
                            rhs=buf[:, bbase + t, 0:feat],
                            start=False, stop=True)
                    # leaky = max(0.2*u, u); DVE may read only one PSUM
                    # operand, so Scalar scales into SBUF first
                    us = wk.tile([P, CH, feat], dt_e, tag="us")
                    nc.scalar.activation(
                        out=us[:, 0:tcnt, :], in_=u_ps[:, 0:tcnt, :],
                        func=mybir.ActivationFunctionType.Copy,
                        scale=NEG_SLOPE)
                    nc.vector.tensor_tensor(
                        out=t_lr[:, boff:boff + tcnt, :],
                        in0=us[:, 0:tcnt, :],
                        in1=u_ps[:, 0:tcnt, :], op=mybir.AluOpType.max)
                # whole-group ops on the unified [A|B] tile
                tp = wk.tile([P, NT, feat], dt_e, tag="tp")
                nc.vector.tensor_tensor(
                    out=tp[:], in0=t_lr[:],
                    in1=c_attn[:].unsqueeze(1).to_broadcast([P, NT, feat]),
                    op=mybir.AluOpType.mult)
                lg = sm.tile([P, NT * nheads], F32, tag="lg")
                nc.vector.tensor_reduce(
                    out=lg[:].rearrange("p (n h) -> p n h", n=NT),
                    in_=tp[:].rearrange("p n (h d) -> p n h d", h=nheads),
                    axis=mybir.AxisListType.X, op=mybir.AluOpType.add)
                w = wk.tile([P, NT, wcols], dt_e, tag="w")
                nc.scalar.activation(
                    out=w[:, :, feat:wcols],
                    in_=lg[:].rearrange("p (n h) -> p n h", n=NT),
                    func=mybir.ActivationFunctionType.Exp)
                for (buf, bbase, tcnt, boff) in (
                    (gA, i * T_A, T_A, 0),
                    (gB, i * T_B, T_B, T_A),
                ):
                    fs_ch = buf[:, bbase:bbase + tcnt, 0:feat]
                    nc.vector.tensor_tensor(
                        out=w[:, boff:boff + tcnt, 0:feat].rearrange(
                            "p n (h d) -> p n h d", h=nheads),
                        in0=fs_ch.rearrange("p n (h d) -> p n h d",
                                            h=nheads),
                        in1=w[:, boff:boff + tcnt, feat:wcols].unsqueeze(3)
                            .to_broadcast([P, tcnt, nheads, hdim]),
                        op=mybir.AluOpType.mult)
                for t in range(NT):
                    nc.tensor.matmul(
                        out=psg[:], lhsT=soh[:, i * NT + t, :],
                        rhs=w[:, t, :],
                        start=(t == 0), stop=(t == NT - 1))

                # normalize group g
                den = sm.tile([P, nheads], F32, tag="den")
                nc.vector.tensor_scalar_add(
                    out=den[:], in0=psg[:, feat:wcols], scalar1=DEN_EPS)
                denr = sm.tile([P, nheads], F32, tag="denr")
                nc.vector.reciprocal(out=denr[:], in_=den[:])
                if layer == 1:
                    h_g = wk.tile([P, feat], BF16, tag="hg")
                    nc.vector.scalar_tensor_tensor(
                        out=h_g[:].rearrange("p (h d) -> p h d", h=nheads),
                        in0=psg[:, 0:feat].rearrange("p (h d) -> p h d",
                                                     h=nheads),
                        scalar=0.0, op0=mybir.AluOpType.max,
                        in1=denr[:].unsqueeze(2).to_broadcast(
                            [P, nheads, hdim]),
                        op1=mybir.AluOpType.mult)
                    # transpose into the feat-major layer-1 output
                    ps_t = pt.tile([P, P], BF16, tag="pst")
                    nc.tensor.transpose(out=ps_t[:], in_=h_g[:],
                                        identity=c_ident[:])
                    nc.scalar.activation(
                        out=c_hT[:, g * P:(g + 1) * P], in_=ps_t[:],
                        func=mybir.ActivationFunctionType.Copy)
                    # layer-2 tables for this group
                    ps2 = pt.tile([P, 2 * cfg.OUT], F32, tag="ps2")
                    nc.tensor.matmul(out=ps2[:],
                                     lhsT=c_hT[:, g * P:(g + 1) * P],
                                     rhs=c_W2[:], start=True, stop=True)
                    sb2 = wk.tile([P, 2 * cfg.OUT], BF16, tag="sb2")
                    nc.vector.tensor_tensor(out=sb2[:], in0=ps2[:],
                                            in1=c_b2r[:],
                                            op=mybir.AluOpType.add)
                    nc.sync.dma_start(out=fs2cat[g * P:(g + 1) * P, :],
                                      in_=sb2[:])
                else:
                    o_g = wk.tile([P, feat], F32, tag="og")
                    nc.vector.tensor_tensor(
                        out=o_g[:], in0=psg[:, 0:feat],
                        in1=denr[:].to_broadcast([P, feat]),
                        op=mybir.AluOpType.mult)
                    nc.sync.dma_start(out=out_local[g * P:(g + 1) * P, :],
                                      in_=o_g[:])


def assemble(results, cfg: Cfg):
    outs = [np.asarray(results[c]["out_local"])[:cfg.NLOC]
            for c in range(cfg.NC)]
    return np.concatenate(outs, axis=0).astype(np.float32)


def kernel(**inputs):
    from concourse.bass_utils import run_bass_kernel_spmd
    cfg = Cfg()
    in_maps = preprocess(inputs, cfg)
    nc = build_program(cfg, debug=not axon_active())
    res = run_bass_kernel_spmd(nc, in_maps, list(range(cfg.NC)))
    return assemble(res.results, cfg)

